# revision 1
# baseline (speedup 1.0000x reference)
"""Trainium2 Bass kernel for nn_AttentionDeduplicate (B=2, Q=K=512, T=128).

Math (identical values to the reference, restructured for the hardware):
  key   = ktok @ Wk.T ; query = qtok @ Wq.T
  sim[k] = kn_k^T G kn_k with G = sum_j kn_j kn_j^T  (Gram over T=128 dims,
           kn = key/||key||) -- avoids the [B,K,K] cosine matrix entirely.
  Per (b,k):  L[s,q] = sum_t Wal[s,t]*key[k,t]*query[q,t]
              done as one [128x128]@[128x512] matmul with the stationary
              operand lhsT_k = WalT * keycol_k (per-partition scale).
  swishmax without the max-subtraction:  u = L*exp(L),
              S = u / (sum_q |u| + sim*e^M),  e^M = max_q exp(L)
        (algebraically equal to the reference's x*exp(x-max)/shrink form;
         |L| <= ~8 for these inputs so exp(L) is safe in fp32)
  value_sum^T = sum_k diag(v_k/d_k) @ u_k  -- K-reduction runs on the
              TensorEngine via diagonal matmuls accumulating in PSUM.
  out = value_sum @ Wvo.T applied per-core; the 4 cores of each batch
        element return partial outputs that the host sums (projection is
        linear so it commutes with the K-partial sum).

Sharding: 8 cores = 2 batches x 4 key-chunks of 128. SPMD: every core runs
the same program; the host rotates the key axis per core so that each
core's local 128 keys are columns 0:128.
"""

import numpy as np
from contextlib import ExitStack

import concourse.bass as bass
import concourse.tile as tile
from concourse import bacc, mybir
from concourse.bass_utils import run_bass_kernel_spmd

F32 = mybir.dt.float32
AF = mybir.ActivationFunctionType
ALU = mybir.AluOpType
AX = mybir.AxisListType

B, Q, K, T = 2, 512, 512, 128
NCORES = 8
KLOC = K // 4     # keys per core
GROUP = 2         # k's fused per DVE/ACT group (PSUM: 3 L-tiles x 2 banks + acc)

_cache = {}


def _build_program():
    nc = bacc.Bacc("TRN2", target_bir_lowering=False, debug=False)

    qT = nc.dram_tensor("qT", [T, Q], F32, kind="ExternalInput").ap()
    kT = nc.dram_tensor("kT", [T, K], F32, kind="ExternalInput").ap()
    WqT = nc.dram_tensor("WqT", [T, T], F32, kind="ExternalInput").ap()
    WkT = nc.dram_tensor("WkT", [T, T], F32, kind="ExternalInput").ap()
    WalT = nc.dram_tensor("WalT", [T, T], F32, kind="ExternalInput").ap()
    WvaT = nc.dram_tensor("WvaT", [T, T], F32, kind="ExternalInput").ap()
    WvoT = nc.dram_tensor("WvoT", [T, T], F32, kind="ExternalInput").ap()
    ident = nc.dram_tensor("ident", [T, T], F32, kind="ExternalInput").ap()
    outT = nc.dram_tensor("outT", [T, Q], F32, kind="ExternalOutput").ap()

    with tile.TileContext(nc) as tc, ExitStack() as ctx:
        consts = ctx.enter_context(tc.tile_pool(name="consts", bufs=1))
        accp = ctx.enter_context(tc.tile_pool(name="accp", bufs=1, space="PSUM"))

        WalT_s = consts.tile([T, T], F32, tag="WalT_s")
        WvoT_s = consts.tile([T, T], F32, tag="WvoT_s")
        ident_s = consts.tile([T, T], F32, tag="ident_s")
        queryT = consts.tile([T, Q], F32, tag="queryT")
        keyT = consts.tile([T, K], F32, tag="keyT")
        vT = consts.tile([T, KLOC], F32, tag="vT")
        simb = consts.tile([T, KLOC], F32, tag="simb")

        nc.sync.dma_start(WalT_s[:], WalT)
        nc.sync.dma_start(WvoT_s[:], WvoT)
        nc.sync.dma_start(ident_s[:], ident)

        acc = accp.tile([T, Q], F32, tag="acc")

        # ---------------- setup: projections, Gram, sim ----------------
        with tc.tile_pool(name="sset", bufs=1) as ss, \
             tc.tile_pool(name="pset", bufs=2, space="PSUM") as ps:
            qT_s = ss.tile([T, Q], F32, tag="qT_s")
            nc.sync.dma_start(qT_s[:], qT)
            kT_s = ss.tile([T, K], F32, tag="kT_s")
            nc.sync.dma_start(kT_s[:], kT)
            WqT_s = ss.tile([T, T], F32, tag="WqT_s")
            nc.sync.dma_start(WqT_s[:], WqT)
            WkT_s = ss.tile([T, T], F32, tag="WkT_s")
            nc.sync.dma_start(WkT_s[:], WkT)
            WvaT_s = ss.tile([T, T], F32, tag="WvaT_s")
            nc.sync.dma_start(WvaT_s[:], WvaT)

            p1 = ps.tile([T, Q], F32, tag="pbig")
            nc.tensor.matmul(p1[:], WqT_s[:], qT_s[:], start=True, stop=True)
            nc.scalar.copy(queryT[:], p1[:])
            p2 = ps.tile([T, K], F32, tag="pbig")
            nc.tensor.matmul(p2[:], WkT_s[:], kT_s[:], start=True, stop=True)
            nc.scalar.copy(keyT[:], p2[:])
            p3 = ps.tile([T, KLOC], F32, tag="pbig")
            nc.tensor.matmul(p3[:], WvaT_s[:], keyT[:, 0:KLOC], start=True, stop=True)
            nc.scalar.copy(vT[:], p3[:])

            # key in [k,t] layout (4 chunks), squared norms, 1/n2, kn-scaled
            key_kt = ss.tile([T, 4, T], F32, tag="key_kt")
            kns = ss.tile([T, 4, T], F32, tag="kns")
            rn2 = ss.tile([T, 4], F32, tag="rn2")
            n2 = ss.tile([T, 4], F32, tag="n2")
            sqd = ss.tile([T, T], F32, tag="sqd")
            for c in range(4):
                pk = ps.tile([T, T], F32, tag="pbig")
                nc.tensor.matmul(pk[:], kT_s[:, c * T:(c + 1) * T], WkT_s[:],
                                 start=True, stop=True)
                nc.scalar.copy(key_kt[:, c, :], pk[:])
                nc.scalar.activation(sqd[:], key_kt[:, c, :], AF.Square,
                                     accum_out=n2[:, c:c + 1])
                nc.vector.reciprocal(rn2[:, c:c + 1], n2[:, c:c + 1])
                nc.vector.tensor_scalar(kns[:, c, :], key_kt[:, c, :],
                                        rn2[:, c:c + 1], None, ALU.mult)
            Gps = ps.tile([T, T], F32, tag="pG")
            for c in range(4):
                nc.tensor.matmul(Gps[:], kns[:, c, :], key_kt[:, c, :],
                                 start=(c == 0), stop=(c == 3))
            G_s = ss.tile([T, T], F32, tag="G_s")
            nc.scalar.copy(G_s[:], Gps[:])

            # sim for the local chunk only (columns 0:KLOC)
            simc = ss.tile([T, 1], F32, tag="simc")
            sttd = ss.tile([T, T], F32, tag="sttd")
            ph = ps.tile([T, T], F32, tag="pbig")
            nc.tensor.matmul(ph[:], keyT[:, 0:T], G_s[:], start=True, stop=True)
            nc.vector.scalar_tensor_tensor(sttd[:], ph[:], rn2[:, 0:1],
                                           key_kt[:, 0, :], ALU.mult, ALU.mult,
                                           accum_out=simc[:])
            # transpose sim column -> row, then broadcast across partitions
            prow = ps.tile([1, KLOC], F32, tag="prow")
            nc.tensor.matmul(prow[:], simc[:], ident_s[:], start=True, stop=True)
            simrow = ss.tile([1, KLOC], F32, tag="simrow")
            nc.scalar.copy(simrow[:], prow[:])
            onesr = ss.tile([1, T], F32, tag="onesr")
            nc.vector.memset(onesr[:], 1.0)
            pb = ps.tile([T, KLOC], F32, tag="pbig")
            nc.tensor.matmul(pb[:], onesr[:], simrow[:], start=True, stop=True)
            nc.scalar.copy(simb[:], pb[:])

        # ---------------- main loop over local keys ----------------
        NG = KLOC // GROUP
        with tc.tile_pool(name="lhs", bufs=4) as lhsp, \
             tc.tile_pool(name="ebuf", bufs=3) as epool, \
             tc.tile_pool(name="ubuf", bufs=3) as upool, \
             tc.tile_pool(name="absd", bufs=2) as apool, \
             tc.tile_pool(name="stats", bufs=4) as st, \
             tc.tile_pool(name="diag", bufs=4) as dgp, \
             tc.tile_pool(name="Lps", bufs=3, space="PSUM") as Lp:
            for g in range(NG):
                j0 = g * GROUP
                Lbig = Lp.tile([T, GROUP * Q], F32, tag="L")
                for i in range(GROUP):
                    lh = lhsp.tile([T, T], F32, tag="lh")
                    nc.vector.tensor_scalar(lh[:], WalT_s[:],
                                            keyT[:, j0 + i:j0 + i + 1], None,
                                            ALU.mult)
                    nc.tensor.matmul(Lbig[:, i * Q:(i + 1) * Q], lh[:],
                                     queryT[:], start=True, stop=True)
                e = epool.tile([T, GROUP * Q], F32, tag="e")
                nc.scalar.activation(e[:], Lbig[:], AF.Exp)
                u = upool.tile([T, GROUP * Q], F32, tag="u")
                nc.vector.tensor_tensor(u[:], Lbig[:], e[:], op=ALU.mult)
                sumabs = st.tile([T, GROUP], F32, tag="sumabs")
                for i in range(GROUP):
                    absd = apool.tile([T, Q], F32, tag="absd")
                    nc.scalar.activation(absd[:], u[:, i * Q:(i + 1) * Q],
                                         AF.Abs, accum_out=sumabs[:, i:i + 1])
                emax = st.tile([T, GROUP], F32, tag="emax")
                nc.vector.tensor_reduce(
                    emax[:], e[:].rearrange("p (g q) -> p g q", g=GROUP),
                    axis=AX.X, op=ALU.max)
                d1 = st.tile([T, GROUP], F32, tag="d1")
                nc.vector.tensor_tensor(d1[:], emax[:],
                                        simb[:, j0:j0 + GROUP], op=ALU.mult)
                d2 = st.tile([T, GROUP], F32, tag="d2")
                nc.vector.tensor_tensor(d2[:], d1[:], sumabs[:], op=ALU.add)
                rd = st.tile([T, GROUP], F32, tag="rd")
                nc.vector.reciprocal(rd[:], d2[:])
                fcol = st.tile([T, GROUP], F32, tag="fcol")
                nc.vector.tensor_tensor(fcol[:], rd[:],
                                        vT[:, j0:j0 + GROUP], op=ALU.mult)
                for i in range(GROUP):
                    j = j0 + i
                    dg = dgp.tile([T, T], F32, tag="dg")
                    nc.vector.tensor_scalar(dg[:], ident_s[:],
                                            fcol[:, i:i + 1], None, ALU.mult)
                    nc.tensor.matmul(acc[:], dg[:], u[:, i * Q:(i + 1) * Q],
                                     start=(j == 0), stop=(j == KLOC - 1))

        # ---------------- final projection ----------------
        with tc.tile_pool(name="fin", bufs=1) as fp, \
             tc.tile_pool(name="fps", bufs=1, space="PSUM") as fps:
            accS = fp.tile([T, Q], F32, tag="accS")
            nc.scalar.copy(accS[:], acc[:])
            po = fps.tile([T, Q], F32, tag="po")
            nc.tensor.matmul(po[:], WvoT_s[:], accS[:], start=True, stop=True)
            outS = fp.tile([T, Q], F32, tag="outS")
            nc.scalar.copy(outS[:], po[:])
            nc.sync.dma_start(outT, outS[:])

    nc.finalize()
    return nc


def _in_maps(query_tokens, key_tokens, Wk, Wq, Wva, Wal, Wvo):
    f = np.float32
    wts = {
        "WqT": np.ascontiguousarray(Wq.T, dtype=f),
        "WkT": np.ascontiguousarray(Wk.T, dtype=f),
        "WalT": np.ascontiguousarray(Wal.T, dtype=f),
        "WvaT": np.ascontiguousarray(Wva.T, dtype=f),
        "WvoT": np.ascontiguousarray(Wvo.T, dtype=f),
        "ident": np.eye(T, dtype=f),
    }
    maps = []
    for c in range(NCORES):
        b, r = c // 4, c % 4
        order = (np.arange(K) + r * KLOC) % K
        maps.append({
            "qT": np.ascontiguousarray(np.asarray(query_tokens)[b].T, dtype=f),
            "kT": np.ascontiguousarray(np.asarray(key_tokens)[b][order].T, dtype=f),
            **wts,
        })
    return maps


def kernel(query_tokens, key_tokens, Wk, Wq, Wva, Wal, Wvo):
    if "nc" not in _cache:
        _cache["nc"] = _build_program()
    nc = _cache["nc"]
    maps = _in_maps(query_tokens, key_tokens, Wk, Wq, Wva, Wal, Wvo)
    res = run_bass_kernel_spmd(nc, maps, core_ids=list(range(NCORES)))
    parts = [r["outT"] for r in res.results]
    out = np.stack(
        [(parts[4 * b] + parts[4 * b + 1] + parts[4 * b + 2] + parts[4 * b + 3]).T
         for b in range(B)]
    ).astype(np.float32)
    return out


# revision 12
# speedup vs baseline: 2.1740x; 2.1740x over previous
"""Trainium2 Bass kernel for nn_AttentionDeduplicate (B=2, Q=K=512, T=128).

Math (identical values to the reference, restructured for the hardware):
  key   = ktok @ Wk.T ; query = qtok @ Wq.T
  sim[k] = kn_k^T G kn_k with G = sum_j kn_j kn_j^T  (Gram over T=128 dims,
           kn = key/||key||) -- avoids the [B,K,K] cosine matrix entirely.
  Per (b,k):  L[s,q] = sum_t Wal[s,t]*key[k,t]*query[q,t]
              done as one [128x128]@[128x512] matmul with the stationary
              operand lhsT_k = WalT * keycol_k (per-partition scale).
  swishmax without the max-subtraction:  u = L*exp(L),
              S = u / (sum_q |u| + sim*e^M),  e^M = max_q exp(L)
        (algebraically equal to the reference's x*exp(x-max)/shrink form;
         |L| <= ~8 for these inputs so exp(L) is safe in fp32)
  value_sum^T = sum_k diag(v_k/d_k) @ u_k  -- K-reduction runs on the
              TensorEngine via diagonal matmuls accumulating in PSUM.
  out = value_sum @ Wvo.T applied per-core; the 4 cores of each batch
        element return partial outputs that the host sums (projection is
        linear so it commutes with the K-partial sum).

Sharding: 8 cores = 2 batches x 4 key-chunks of 128. SPMD: every core runs
the same program; the host rotates the key axis per core so that each
core's local 128 keys are columns 0:128.
"""

import numpy as np
from contextlib import ExitStack

import concourse.bass as bass
import concourse.tile as tile
from concourse import bacc, mybir
from concourse.bass_utils import run_bass_kernel_spmd

F32 = mybir.dt.float32
BF16 = mybir.dt.bfloat16
AF = mybir.ActivationFunctionType
ALU = mybir.AluOpType
AX = mybir.AxisListType

B, Q, K, T = 2, 512, 512, 128
NCORES = 8
KLOC = K // 4     # keys per core
GROUP = 2         # k's fused per DVE/ACT group (PSUM: 3 L-tiles x 2 banks + acc)

_cache = {}

# stage toggles for cost attribution (all True for the real kernel)
DBG_STAGES = dict(lh=True, mmL=True, exp=True, umul=True, babs=True,
                  bmax=True, tiny=True, diag=True, mmacc=True, setup=True)


def _build_program(dbg=None):
    st = dict(DBG_STAGES)
    if dbg:
        st.update(dbg)
    nc = bacc.Bacc("TRN2", target_bir_lowering=False, debug=False)

    qT = nc.dram_tensor("qT", [T, Q], F32, kind="ExternalInput").ap()
    kT = nc.dram_tensor("kT", [T, K], F32, kind="ExternalInput").ap()
    WqT = nc.dram_tensor("WqT", [T, T], F32, kind="ExternalInput").ap()
    WkT = nc.dram_tensor("WkT", [T, T], F32, kind="ExternalInput").ap()
    WalT = nc.dram_tensor("WalT", [T, T], F32, kind="ExternalInput").ap()
    WvaT = nc.dram_tensor("WvaT", [T, T], F32, kind="ExternalInput").ap()
    WvoT = nc.dram_tensor("WvoT", [T, T], F32, kind="ExternalInput").ap()
    ident = nc.dram_tensor("ident", [T, T], F32, kind="ExternalInput").ap()
    outT = nc.dram_tensor("outT", [T, Q], F32, kind="ExternalOutput").ap()

    with tile.TileContext(nc) as tc, ExitStack() as ctx:
        consts = ctx.enter_context(tc.tile_pool(name="consts", bufs=1))
        accp = ctx.enter_context(tc.tile_pool(name="accp", bufs=1, space="PSUM"))

        WalT_s = consts.tile([T, T], F32, tag="WalT_s")
        WvoT_s = consts.tile([T, T], F32, tag="WvoT_s")
        ident_s = consts.tile([T, T], F32, tag="ident_s")
        queryT = consts.tile([T, Q], BF16, tag="queryT")
        keyT = consts.tile([T, K], F32, tag="keyT")
        vT = consts.tile([T, KLOC], F32, tag="vT")
        simb = consts.tile([T, KLOC], F32, tag="simb")

        nc.sync.dma_start(WalT_s[:], WalT)
        nc.sync.dma_start(WvoT_s[:], WvoT)
        nc.sync.dma_start(ident_s[:], ident)

        acc = accp.tile([T, Q], F32, tag="acc")

        # ---------------- setup: projections, Gram, sim ----------------
        with tc.tile_pool(name="sset", bufs=1) as ss, \
             tc.tile_pool(name="pset", bufs=2, space="PSUM") as ps:
            qT_s = ss.tile([T, Q], F32, tag="qT_s")
            nc.sync.dma_start(qT_s[:], qT)
            kT_s = ss.tile([T, K], F32, tag="kT_s")
            nc.sync.dma_start(kT_s[:], kT)
            WqT_s = ss.tile([T, T], F32, tag="WqT_s")
            nc.sync.dma_start(WqT_s[:], WqT)
            WkT_s = ss.tile([T, T], F32, tag="WkT_s")
            nc.sync.dma_start(WkT_s[:], WkT)
            WvaT_s = ss.tile([T, T], F32, tag="WvaT_s")
            nc.sync.dma_start(WvaT_s[:], WvaT)

            p1 = ps.tile([T, Q], F32, tag="pbig")
            nc.tensor.matmul(p1[:], WqT_s[:], qT_s[:], start=True, stop=True)
            nc.scalar.copy(queryT[:], p1[:])
            p2 = ps.tile([T, K], F32, tag="pbig")
            nc.tensor.matmul(p2[:], WkT_s[:], kT_s[:], start=True, stop=True)
            nc.scalar.copy(keyT[:], p2[:])
            p3 = ps.tile([T, KLOC], F32, tag="pbig")
            nc.tensor.matmul(p3[:], WvaT_s[:], keyT[:, 0:KLOC], start=True, stop=True)
            nc.scalar.copy(vT[:], p3[:])

            # key in [k,t] layout (4 chunks), squared norms, 1/n2, kn-scaled
            key_kt = ss.tile([T, 4, T], F32, tag="key_kt")
            kns = ss.tile([T, 4, T], F32, tag="kns")
            rn2 = ss.tile([T, 4], F32, tag="rn2")
            n2 = ss.tile([T, 4], F32, tag="n2")
            sqd = ss.tile([T, T], F32, tag="sqd")
            for c in range(4):
                pk = ps.tile([T, T], F32, tag="pbig")
                nc.tensor.matmul(pk[:], kT_s[:, c * T:(c + 1) * T], WkT_s[:],
                                 start=True, stop=True)
                nc.scalar.copy(key_kt[:, c, :], pk[:])
                nc.scalar.activation(sqd[:], key_kt[:, c, :], AF.Square,
                                     accum_out=n2[:, c:c + 1])
                nc.vector.reciprocal(rn2[:, c:c + 1], n2[:, c:c + 1])
                nc.vector.tensor_scalar(kns[:, c, :], key_kt[:, c, :],
                                        rn2[:, c:c + 1], None, ALU.mult)
            Gps = ps.tile([T, T], F32, tag="pG")
            for c in range(4):
                nc.tensor.matmul(Gps[:], kns[:, c, :], key_kt[:, c, :],
                                 start=(c == 0), stop=(c == 3))
            G_s = ss.tile([T, T], F32, tag="G_s")
            nc.scalar.copy(G_s[:], Gps[:])

            # sim for the local chunk only (columns 0:KLOC)
            simc = ss.tile([T, 1], F32, tag="simc")
            sttd = ss.tile([T, T], F32, tag="sttd")
            ph = ps.tile([T, T], F32, tag="pbig")
            nc.tensor.matmul(ph[:], keyT[:, 0:T], G_s[:], start=True, stop=True)
            nc.vector.scalar_tensor_tensor(sttd[:], ph[:], rn2[:, 0:1],
                                           key_kt[:, 0, :], ALU.mult, ALU.mult,
                                           accum_out=simc[:])
            # transpose sim column -> row, then broadcast across partitions
            prow = ps.tile([1, KLOC], F32, tag="prow")
            nc.tensor.matmul(prow[:], simc[:], ident_s[:], start=True, stop=True)
            simrow = ss.tile([1, KLOC], F32, tag="simrow")
            nc.scalar.copy(simrow[:], prow[:])
            onesr = ss.tile([1, T], F32, tag="onesr")
            nc.vector.memset(onesr[:], 1.0)
            pb = ps.tile([T, KLOC], F32, tag="pbig")
            nc.tensor.matmul(pb[:], onesr[:], simrow[:], start=True, stop=True)
            nc.scalar.copy(simb[:], pb[:])

        # ---------------- main loop over local keys ----------------
        # Block-pipelined: BLOCK groups of GROUP keys each per block.
        # Engine roles: PE mmL/mmacc, ACT exp/abs, DVE umul/max/denominator,
        # GPSIMD lh/diag builds. The acc matmuls trail by one block and the
        # lh builds lead by one block so no engine queue head-blocks.
        NG = KLOC // GROUP
        BLOCK = 4
        NB = NG // BLOCK
        BK = BLOCK * GROUP   # keys per block
        live = {}

        lh_eng = nc.gpsimd
        dg_eng = nc.gpsimd

        def emit_lh(b):
            # stationary operands for block b's L matmuls
            lhs = []
            for i in range(BK):
                j = b * BK + i
                lh = lhsp.tile([T, T], BF16, tag="lh")
                lw = T if st["lh"] else 8
                lh_eng.tensor_scalar(lh[:, 0:lw], WalT_s[:, 0:lw],
                                     keyT[:, j:j + 1], None, ALU.mult)
                lhs.append(lh)
            live[("lh", b)] = lhs

        def emit_block(b):
            lhs = live.pop(("lh", b))
            Ls, es, us = [], [], []
            for g in range(BLOCK):
                Lbig = Lp.tile([T, GROUP * Q], F32, tag="L")
                for i in range(GROUP):
                    mw = Q if st["mmL"] else 8
                    nc.tensor.matmul(Lbig[:, i * Q:i * Q + mw],
                                     lhs[g * GROUP + i][:],
                                     queryT[:, 0:mw], start=True, stop=True)
                Ls.append(Lbig)
                e = epool.tile([T, GROUP * Q], BF16, tag="e")
                ew = GROUP * Q if st["exp"] else 8
                nc.scalar.activation(e[:, 0:ew], Lbig[:, 0:ew], AF.Exp)
                es.append(e)
            for g in range(BLOCK):
                u = upool.tile([T, GROUP * Q], BF16, tag="u")
                uw = GROUP * Q if st["umul"] else 8
                nc.vector.tensor_tensor(u[:, 0:uw], Ls[g][:, 0:uw],
                                        es[g][:, 0:uw], op=ALU.mult)
                us.append(u)
            sumabs = st_pool.tile([T, BK], F32, tag="sumabs")
            aw = Q if st["babs"] else 8
            for g in range(BLOCK):
                for i in range(GROUP):
                    if g == 0 and i == 0:
                        # one |u| row-sum per block runs on DVE to balance
                        # the ACT/DVE load
                        nc.vector.tensor_reduce(
                            sumabs[:, 0:1],
                            us[0][:, 0:aw].rearrange("p (g q) -> p g q", g=1),
                            axis=AX.X, op=ALU.add, apply_absolute_value=True)
                        continue
                    absd = apool.tile([T, Q], BF16, tag="absd")
                    nc.scalar.activation(absd[:, 0:aw],
                                         us[g][:, i * Q:i * Q + aw], AF.Abs,
                                         accum_out=sumabs[:, g * GROUP + i:
                                                          g * GROUP + i + 1])
            emax = st_pool.tile([T, BK], F32, tag="emax")
            xw = Q if st["bmax"] else 8
            for g in range(BLOCK):
                nc.vector.tensor_reduce(
                    emax[:, g * GROUP:(g + 1) * GROUP],
                    es[g][:].rearrange("p (g q) -> p g q", g=GROUP)[:, :, 0:xw],
                    axis=AX.X, op=ALU.max)
            # lh for the next block leads the denominator work so the
            # GPSIMD queue never blocks next block's matmuls
            if b + 1 < NB:
                emit_lh(b + 1)
            j0 = b * BK
            fcol = st_pool.tile([T, BK], F32, tag="fcol")
            if st["tiny"]:
                d1 = st_pool.tile([T, BK], F32, tag="d1")
                nc.vector.tensor_tensor(d1[:], emax[:],
                                        simb[:, j0:j0 + BK], op=ALU.mult)
                d2 = st_pool.tile([T, BK], F32, tag="d2")
                nc.vector.tensor_tensor(d2[:], d1[:], sumabs[:], op=ALU.add)
                rd = st_pool.tile([T, BK], F32, tag="rd")
                nc.vector.reciprocal(rd[:], d2[:])
                nc.vector.tensor_tensor(fcol[:], rd[:],
                                        vT[:, j0:j0 + BK], op=ALU.mult)
            dgs = []
            for i in range(BK):
                dg = dgp.tile([T, T], BF16, tag="dg")
                dw = T if st["diag"] else 8
                dg_eng.tensor_scalar(dg[:, 0:dw], ident_s[:, 0:dw],
                                     fcol[:, i:i + 1], None, ALU.mult)
                dgs.append(dg)
            live[b] = dict(us=us, dgs=dgs)

        def emit_acc(b):
            us, dgs = live[b]["us"], live[b]["dgs"]
            for g in range(BLOCK):
                for i in range(GROUP):
                    j = b * BK + g * GROUP + i
                    aw2 = Q if st["mmacc"] else 8
                    nc.tensor.matmul(acc[:, 0:aw2], dgs[g * GROUP + i][:],
                                     us[g][:, i * Q:i * Q + aw2],
                                     start=(j == 0), stop=(j == KLOC - 1))
            del live[b]

        with tc.tile_pool(name="lhs", bufs=2 * BK + 2) as lhsp, \
             tc.tile_pool(name="ebuf", bufs=BLOCK + 2) as epool, \
             tc.tile_pool(name="ubuf", bufs=2 * BLOCK + 2) as upool, \
             tc.tile_pool(name="absd", bufs=3) as apool, \
             tc.tile_pool(name="stats", bufs=3) as st_pool, \
             tc.tile_pool(name="diag", bufs=2 * BK + 2) as dgp, \
             tc.tile_pool(name="Lps", bufs=3, space="PSUM") as Lp:
            emit_lh(0)
            for b in range(NB):
                emit_block(b)
                if b >= 1:
                    emit_acc(b - 1)
            emit_acc(NB - 1)

        # ---------------- final projection ----------------
        with tc.tile_pool(name="fin", bufs=1) as fp, \
             tc.tile_pool(name="fps", bufs=1, space="PSUM") as fps:
            accS = fp.tile([T, Q], F32, tag="accS")
            nc.scalar.copy(accS[:], acc[:])
            po = fps.tile([T, Q], F32, tag="po")
            nc.tensor.matmul(po[:], WvoT_s[:], accS[:], start=True, stop=True)
            outS = fp.tile([T, Q], F32, tag="outS")
            nc.scalar.copy(outS[:], po[:])
            nc.sync.dma_start(outT, outS[:])

    nc.finalize()
    return nc


def _in_maps(query_tokens, key_tokens, Wk, Wq, Wva, Wal, Wvo):
    f = np.float32
    wts = {
        "WqT": np.ascontiguousarray(Wq.T, dtype=f),
        "WkT": np.ascontiguousarray(Wk.T, dtype=f),
        "WalT": np.ascontiguousarray(Wal.T, dtype=f),
        "WvaT": np.ascontiguousarray(Wva.T, dtype=f),
        "WvoT": np.ascontiguousarray(Wvo.T, dtype=f),
        "ident": np.eye(T, dtype=f),
    }
    maps = []
    for c in range(NCORES):
        b, r = c // 4, c % 4
        order = (np.arange(K) + r * KLOC) % K
        maps.append({
            "qT": np.ascontiguousarray(np.asarray(query_tokens)[b].T, dtype=f),
            "kT": np.ascontiguousarray(np.asarray(key_tokens)[b][order].T, dtype=f),
            **wts,
        })
    return maps


def kernel(query_tokens, key_tokens, Wk, Wq, Wva, Wal, Wvo):
    if "nc" not in _cache:
        _cache["nc"] = _build_program()
    nc = _cache["nc"]
    maps = _in_maps(query_tokens, key_tokens, Wk, Wq, Wva, Wal, Wvo)
    res = run_bass_kernel_spmd(nc, maps, core_ids=list(range(NCORES)))
    parts = [r["outT"] for r in res.results]
    out = np.stack(
        [(parts[4 * b] + parts[4 * b + 1] + parts[4 * b + 2] + parts[4 * b + 3]).T
         for b in range(B)]
    ).astype(np.float32)
    return out


# revision 17
# speedup vs baseline: 2.3115x; 1.0632x over previous
"""Trainium2 Bass kernel for nn_AttentionDeduplicate (B=2, Q=K=512, T=128).

Math (identical values to the reference, restructured for the hardware):
  key   = ktok @ Wk.T ; query = qtok @ Wq.T
  sim[k] = kn_k^T G kn_k with G = sum_j kn_j kn_j^T  (Gram over T=128 dims,
           kn = key/||key||) -- avoids the [B,K,K] cosine matrix entirely.
  Per (b,k):  L[s,q] = sum_t Wal[s,t]*key[k,t]*query[q,t]
              done as one [128x128]@[128x512] matmul with the stationary
              operand lhsT_k = WalT * keycol_k (per-partition scale).
  swishmax without the max-subtraction:  u = L*exp(L),
              S = u / (sum_q |u| + sim*e^M),  e^M = max_q exp(L)
        (algebraically equal to the reference's x*exp(x-max)/shrink form;
         |L| <= ~8 for these inputs so exp(L) is safe in fp32)
  value_sum^T = sum_k diag(v_k/d_k) @ u_k  -- K-reduction runs on the
              TensorEngine via diagonal matmuls accumulating in PSUM.
  out = value_sum @ Wvo.T applied per-core; the 4 cores of each batch
        element return partial outputs that the host sums (projection is
        linear so it commutes with the K-partial sum).

Sharding: 8 cores = 2 batches x 4 key-chunks of 128. SPMD: every core runs
the same program; the host rotates the key axis per core so that each
core's local 128 keys are columns 0:128.
"""

import numpy as np
from contextlib import ExitStack

import concourse.bass as bass
import concourse.tile as tile
from concourse import bacc, mybir
from concourse.bass_utils import run_bass_kernel_spmd

F32 = mybir.dt.float32
BF16 = mybir.dt.bfloat16
AF = mybir.ActivationFunctionType
ALU = mybir.AluOpType
AX = mybir.AxisListType

B, Q, K, T = 2, 512, 512, 128
NCORES = 8
KLOC = K // 4     # keys per core
GROUP = 2         # k's fused per DVE/ACT group (PSUM: 3 L-tiles x 2 banks + acc)

_cache = {}

# stage toggles for cost attribution (all True for the real kernel)
DBG_STAGES = dict(lh=True, mmL=True, exp=True, umul=True, babs=True,
                  bmax=True, tiny=True, diag=True, mmacc=True, setup=True)


def _build_program(dbg=None):
    st = dict(DBG_STAGES)
    if dbg:
        st.update(dbg)
    nc = bacc.Bacc("TRN2", target_bir_lowering=False, debug=False)

    qT = nc.dram_tensor("qT", [T, Q], F32, kind="ExternalInput").ap()
    kT = nc.dram_tensor("kT", [T, K], F32, kind="ExternalInput").ap()
    # wpack: WkT | WalT | WqT | WvaT | WvoT | ident  (one DMA)
    wpack = nc.dram_tensor("wpack", [T, 6 * T], F32, kind="ExternalInput").ap()
    outT = nc.dram_tensor("outT", [T, Q], F32, kind="ExternalOutput").ap()

    with tile.TileContext(nc) as tc, ExitStack() as ctx:
        consts = ctx.enter_context(tc.tile_pool(name="consts", bufs=1))
        accp = ctx.enter_context(tc.tile_pool(name="accp", bufs=1, space="PSUM"))

        wp = consts.tile([T, 6 * T], F32, tag="wp")
        nc.sync.dma_start(wp[:], wpack)
        WkT_s = wp[:, 0 * T:1 * T]
        WalT_s = wp[:, 1 * T:2 * T]
        WqT_s = wp[:, 2 * T:3 * T]
        WvaT_s = wp[:, 3 * T:4 * T]
        WvoT_s = wp[:, 4 * T:5 * T]
        ident_s = wp[:, 5 * T:6 * T]
        queryT = consts.tile([T, Q], BF16, tag="queryT")
        keyT = consts.tile([T, K], F32, tag="keyT")
        vT = consts.tile([T, KLOC], F32, tag="vT")
        simb = consts.tile([T, KLOC], F32, tag="simb")

        acc = accp.tile([T, Q], F32, tag="acc")

        # -------- early setup: projections (scoped psum pool) --------
        ss = ctx.enter_context(tc.tile_pool(name="sset", bufs=1))
        with tc.tile_pool(name="pearly", bufs=2, space="PSUM") as ps:
            kT_s = ss.tile([T, K], F32, tag="kT_s")
            nc.sync.dma_start(kT_s[:], kT)
            qT_s = ss.tile([T, Q], F32, tag="qT_s")
            nc.sync.dma_start(qT_s[:], qT)

            p2 = ps.tile([T, K], F32, tag="pbig")
            nc.tensor.matmul(p2[:, 0:T], WkT_s, kT_s[:, 0:T],
                             start=True, stop=True)
            nc.scalar.copy(keyT[:, 0:T], p2[:, 0:T])
            nc.tensor.matmul(p2[:, T:K], WkT_s, kT_s[:, T:K],
                             start=True, stop=True)
            nc.scalar.copy(keyT[:, T:K], p2[:, T:K])
            p1 = ps.tile([T, Q], F32, tag="pbig")
            nc.tensor.matmul(p1[:], WqT_s, qT_s[:], start=True, stop=True)
            nc.scalar.copy(queryT[:], p1[:])
            p3 = ps.tile([T, KLOC], F32, tag="pbig")
            nc.tensor.matmul(p3[:], WvaT_s, keyT[:, 0:KLOC], start=True, stop=True)
            nc.scalar.copy(vT[:], p3[:])

        def emit_sim(simp):
            # Gram + similarity chain; emitted under block 0 so its serial
            # latency hides behind the first block's streaming work.
            key_kt = ss.tile([T, 4, T], F32, tag="key_kt")
            kns = ss.tile([T, 4, T], F32, tag="kns")
            rn2 = ss.tile([T, 4], F32, tag="rn2")
            n2 = ss.tile([T, 4], F32, tag="n2")
            sqd = ss.tile([T, T], F32, tag="sqd")
            for c in range(4):
                pk = simp.tile([T, T], F32, tag="pbig")
                nc.tensor.matmul(pk[:], kT_s[:, c * T:(c + 1) * T], WkT_s,
                                 start=True, stop=True)
                nc.scalar.copy(key_kt[:, c, :], pk[:])
                nc.scalar.activation(sqd[:], key_kt[:, c, :], AF.Square,
                                     accum_out=n2[:, c:c + 1])
                nc.vector.reciprocal(rn2[:, c:c + 1], n2[:, c:c + 1])
                nc.vector.tensor_scalar(kns[:, c, :], key_kt[:, c, :],
                                        rn2[:, c:c + 1], None, ALU.mult)
            # Gram accumulates in the (still unused) acc bank; the first
            # mmacc has start=True which resets the bank afterwards
            for c in range(4):
                nc.tensor.matmul(acc[:, 0:T], kns[:, c, :], key_kt[:, c, :],
                                 start=(c == 0), stop=(c == 3))
            G_s = ss.tile([T, T], F32, tag="G_s")
            nc.scalar.copy(G_s[:], acc[:, 0:T])
            simc = ss.tile([T, 1], F32, tag="simc")
            sttd = ss.tile([T, T], F32, tag="sttd")
            ph = simp.tile([T, T], F32, tag="pbig")
            nc.tensor.matmul(ph[:], keyT[:, 0:T], G_s[:], start=True, stop=True)
            nc.vector.scalar_tensor_tensor(sttd[:], ph[:], rn2[:, 0:1],
                                           key_kt[:, 0, :], ALU.mult, ALU.mult,
                                           accum_out=simc[:])
            # transpose sim column -> row, then broadcast across partitions
            prow = simp.tile([1, KLOC], F32, tag="pbig")
            nc.tensor.matmul(prow[:], simc[:], ident_s, start=True, stop=True)
            simrow = ss.tile([1, KLOC], F32, tag="simrow")
            nc.scalar.copy(simrow[:], prow[:])
            onesr = ss.tile([1, T], F32, tag="onesr")
            nc.vector.memset(onesr[:], 1.0)
            pb = simp.tile([T, KLOC], F32, tag="pbig")
            nc.tensor.matmul(pb[:], onesr[:], simrow[:], start=True, stop=True)
            nc.scalar.copy(simb[:], pb[:])

        # ---------------- main loop over local keys ----------------
        # Block-pipelined: BLOCK groups of GROUP keys each per block.
        # Engine roles: PE mmL/mmacc, ACT exp/abs, DVE umul/max/denominator,
        # GPSIMD lh/diag builds. Emission order per iteration b:
        #   front(b) [mmL/exp/umul/abs/max], lh(b+1), back(b-1) [denom/diag],
        #   acc(b-2) -- so no engine queue ever head-blocks on a younger dep.
        NG = KLOC // GROUP
        BLOCK = 4
        NB = NG // BLOCK
        BK = BLOCK * GROUP   # keys per block
        live = {}

        lh_eng = nc.gpsimd
        dg_eng = nc.gpsimd

        def emit_lh(b):
            lhs = []
            for i in range(BK):
                j = b * BK + i
                lh = lhsp.tile([T, T], BF16, tag="lh")
                lw = T if st["lh"] else 8
                lh_eng.tensor_scalar(lh[:, 0:lw], WalT_s[:, 0:lw],
                                     keyT[:, j:j + 1], None, ALU.mult)
                lhs.append(lh)
            live[("lh", b)] = lhs

        def emit_front(b):
            lhs = live.pop(("lh", b))
            Ls, es, us = [], [], []
            for g in range(BLOCK):
                Lbig = Lp.tile([T, GROUP * Q], F32, tag="L")
                for i in range(GROUP):
                    mw = Q if st["mmL"] else 8
                    nc.tensor.matmul(Lbig[:, i * Q:i * Q + mw],
                                     lhs[g * GROUP + i][:],
                                     queryT[:, 0:mw], start=True, stop=True)
                Ls.append(Lbig)
                e = epool.tile([T, GROUP * Q], BF16, tag="e")
                ew = GROUP * Q if st["exp"] else 8
                nc.scalar.activation(e[:, 0:ew], Lbig[:, 0:ew], AF.Exp)
                es.append(e)
            for g in range(BLOCK):
                u = upool.tile([T, GROUP * Q], BF16, tag="u")
                uw = GROUP * Q if st["umul"] else 8
                nc.vector.tensor_tensor(u[:, 0:uw], Ls[g][:, 0:uw],
                                        es[g][:, 0:uw], op=ALU.mult)
                us.append(u)
            sumabs = st_pool.tile([T, BK], F32, tag="sumabs")
            aw = Q if st["babs"] else 8
            ndve = 1 if (b % 2 == 0) else 2
            for g in range(BLOCK):
                for i in range(GROUP):
                    if i == 0 and g < ndve:
                        # one |u| row-sum per block runs on DVE to balance
                        # the ACT/DVE load
                        nc.vector.tensor_reduce(
                            sumabs[:, g * GROUP:g * GROUP + 1],
                            us[g][:, 0:aw].rearrange("p (g q) -> p g q", g=1),
                            axis=AX.X, op=ALU.add, apply_absolute_value=True)
                        continue
                    absd = apool.tile([T, Q], BF16, tag="absd")
                    nc.scalar.activation(absd[:, 0:aw],
                                         us[g][:, i * Q:i * Q + aw], AF.Abs,
                                         accum_out=sumabs[:, g * GROUP + i:
                                                          g * GROUP + i + 1])
            emax = st_pool.tile([T, BK], F32, tag="emax")
            xw = Q if st["bmax"] else 8
            for g in range(BLOCK):
                # two-phase max: pairwise TT max (2x mode on bf16), then a
                # half-length reduce
                ev = es[g][:].rearrange("p (k h q) -> p k h q", k=GROUP, h=2)
                m1 = mpool.tile([T, GROUP * Q // 2], BF16, tag="m1")
                m1v = m1[:].rearrange("p (k q) -> p k q", k=GROUP)
                nc.vector.tensor_tensor(m1v[:, :, 0:xw // 2],
                                        ev[:, :, 0, 0:xw // 2],
                                        ev[:, :, 1, 0:xw // 2], op=ALU.max)
                nc.vector.tensor_reduce(
                    emax[:, g * GROUP:(g + 1) * GROUP],
                    m1v[:, :, 0:xw // 2], axis=AX.X, op=ALU.max)
            live[b] = dict(us=us, emax=emax, sumabs=sumabs)

        def emit_back(b, final=False):
            j0 = b * BK
            emax, sumabs = live[b]["emax"], live[b]["sumabs"]
            fcol = st_pool.tile([T, BK], F32, tag="fcol")
            tt_eng = nc.vector if final else nc.gpsimd
            if st["tiny"]:
                d1 = st_pool.tile([T, BK], F32, tag="d1")
                tt_eng.tensor_tensor(d1[:], emax[:],
                                     simb[:, j0:j0 + BK], op=ALU.mult)
                d2 = st_pool.tile([T, BK], F32, tag="d2")
                tt_eng.tensor_tensor(d2[:], d1[:], sumabs[:], op=ALU.add)
                rd = st_pool.tile([T, BK], F32, tag="rd")
                nc.vector.reciprocal(rd[:], d2[:])
                tt_eng.tensor_tensor(fcol[:], rd[:],
                                     vT[:, j0:j0 + BK], op=ALU.mult)
            dgs = []
            us = live[b]["us"]
            for i in range(BK):
                dg = dgp.tile([T, T], BF16, tag="dg")
                dw = T if st["diag"] else 8
                eng = nc.vector if final else dg_eng
                eng.tensor_scalar(dg[:, 0:dw], WvoT_s[:, 0:dw],
                                  fcol[:, i:i + 1], None, ALU.mult)
                dgs.append(dg)
                if final:
                    j = b * BK + i
                    aw2 = Q if st["mmacc"] else 8
                    nc.tensor.matmul(acc[:, 0:aw2], dg[:],
                                     us[i // GROUP][:, (i % GROUP) * Q:
                                                    (i % GROUP) * Q + aw2],
                                     start=(j == 0), stop=(j == KLOC - 1))
            live[b]["dgs"] = dgs

        def emit_acc(b):
            us, dgs = live[b]["us"], live[b]["dgs"]
            for g in range(BLOCK):
                for i in range(GROUP):
                    j = b * BK + g * GROUP + i
                    aw2 = Q if st["mmacc"] else 8
                    nc.tensor.matmul(acc[:, 0:aw2], dgs[g * GROUP + i][:],
                                     us[g][:, i * Q:i * Q + aw2],
                                     start=(j == 0), stop=(j == KLOC - 1))
            del live[b]

        with tc.tile_pool(name="lhs", bufs=2 * BK + 2) as lhsp, \
             tc.tile_pool(name="ebuf", bufs=BLOCK + 2) as epool, \
             tc.tile_pool(name="ubuf", bufs=3 * BLOCK + 2) as upool, \
             tc.tile_pool(name="absd", bufs=3) as apool, \
             tc.tile_pool(name="maxb", bufs=2) as mpool, \
             tc.tile_pool(name="stats", bufs=3) as st_pool, \
             tc.tile_pool(name="diag", bufs=2 * BK + 2) as dgp, \
             tc.tile_pool(name="Lps", bufs=3, space="PSUM") as Lp, \
             tc.tile_pool(name="simp", bufs=1, space="PSUM") as simp:
            emit_lh(0)
            for b in range(NB):
                emit_front(b)
                if b == 0:
                    emit_sim(simp)
                if b + 1 < NB:
                    emit_lh(b + 1)
                if b >= 1:
                    emit_back(b - 1)
                if b >= 2:
                    emit_acc(b - 2)
            emit_acc(NB - 2)
            emit_back(NB - 1, final=True)

        # ---------------- final: evacuate acc (already projected) ------
        with tc.tile_pool(name="fin", bufs=1) as fp:
            outS = fp.tile([T, Q], F32, tag="outS")
            nc.scalar.copy(outS[:], acc[:])
            nc.sync.dma_start(outT, outS[:])

    nc.finalize()
    return nc


def _in_maps(query_tokens, key_tokens, Wk, Wq, Wva, Wal, Wvo):
    f = np.float32
    wpack = np.concatenate(
        [np.asarray(w).T.astype(f) for w in (Wk, Wal, Wq, Wva, Wvo)]
        + [np.eye(T, dtype=f)], axis=1)
    wts = {"wpack": np.ascontiguousarray(wpack)}
    maps = []
    for c in range(NCORES):
        b, r = c // 4, c % 4
        order = (np.arange(K) + r * KLOC) % K
        maps.append({
            "qT": np.ascontiguousarray(np.asarray(query_tokens)[b].T, dtype=f),
            "kT": np.ascontiguousarray(np.asarray(key_tokens)[b][order].T, dtype=f),
            **wts,
        })
    return maps


def kernel(query_tokens, key_tokens, Wk, Wq, Wva, Wal, Wvo):
    if "nc" not in _cache:
        _cache["nc"] = _build_program()
    nc = _cache["nc"]
    maps = _in_maps(query_tokens, key_tokens, Wk, Wq, Wva, Wal, Wvo)
    res = run_bass_kernel_spmd(nc, maps, core_ids=list(range(NCORES)))
    parts = [r["outT"] for r in res.results]
    out = np.stack(
        [(parts[4 * b] + parts[4 * b + 1] + parts[4 * b + 2] + parts[4 * b + 3]).T
         for b in range(B)]
    ).astype(np.float32)
    return out


# revision 21
# speedup vs baseline: 2.3621x; 1.0219x over previous
"""Trainium2 Bass kernel for nn_AttentionDeduplicate (B=2, Q=K=512, T=128).

Math (identical values to the reference, restructured for the hardware):
  key   = ktok @ Wk.T ; query = qtok @ Wq.T
  sim[k] = kn_k^T G kn_k with G = sum_j kn_j kn_j^T  (Gram over T=128 dims,
           kn = key/||key||) -- avoids the [B,K,K] cosine matrix entirely.
  Per (b,k):  L[s,q] = sum_t Wal[s,t]*key[k,t]*query[q,t]
              done as one [128x128]@[128x512] matmul with the stationary
              operand lhsT_k = WalT * keycol_k (per-partition scale).
  swishmax without the max-subtraction:  u = L*exp(L),
              S = u / (sum_q |u| + sim*e^M),  e^M = max_q exp(L)
        (algebraically equal to the reference's x*exp(x-max)/shrink form;
         |L| <= ~8 for these inputs so exp(L) is safe in fp32)
  value_sum^T = sum_k diag(v_k/d_k) @ u_k  -- K-reduction runs on the
              TensorEngine via diagonal matmuls accumulating in PSUM.
  out = value_sum @ Wvo.T applied per-core; the 4 cores of each batch
        element return partial outputs that the host sums (projection is
        linear so it commutes with the K-partial sum).

Sharding: 8 cores = 2 batches x 4 key-chunks of 128. SPMD: every core runs
the same program; the host rotates the key axis per core so that each
core's local 128 keys are columns 0:128.
"""

import numpy as np
from contextlib import ExitStack

import concourse.bass as bass
import concourse.tile as tile
from concourse import bacc, mybir
from concourse.bass_utils import run_bass_kernel_spmd

F32 = mybir.dt.float32
BF16 = mybir.dt.bfloat16
AF = mybir.ActivationFunctionType
ALU = mybir.AluOpType
AX = mybir.AxisListType

B, Q, K, T = 2, 512, 512, 128
NCORES = 8
KLOC = K // 4     # keys per core
GROUP = 2         # k's fused per DVE/ACT group (PSUM: 3 L-tiles x 2 banks + acc)

_cache = {}

# stage toggles for cost attribution (all True for the real kernel)
DBG_STAGES = dict(lh=True, mmL=True, exp=True, umul=True, babs=True,
                  bmax=True, tiny=True, diag=True, mmacc=True, setup=True)


def _build_program(dbg=None):
    st = dict(DBG_STAGES)
    if dbg:
        st.update(dbg)
    nc = bacc.Bacc("TRN2", target_bir_lowering=False, debug=False)

    qT = nc.dram_tensor("qT", [T, Q], F32, kind="ExternalInput").ap()
    kT = nc.dram_tensor("kT", [T, K], F32, kind="ExternalInput").ap()
    # wpack: WkT | WalT | WqT | WvaT | WvoT | ident  (one DMA)
    wpack = nc.dram_tensor("wpack", [T, 6 * T], F32, kind="ExternalInput").ap()
    outT = nc.dram_tensor("outT", [T, Q], F32, kind="ExternalOutput").ap()

    with tile.TileContext(nc) as tc, ExitStack() as ctx:
        consts = ctx.enter_context(tc.tile_pool(name="consts", bufs=1))
        accp = ctx.enter_context(tc.tile_pool(name="accp", bufs=1, space="PSUM"))

        wp = consts.tile([T, 6 * T], F32, tag="wp")
        nc.sync.dma_start(wp[:], wpack)
        WkT_s = wp[:, 0 * T:1 * T]
        WalT_s = wp[:, 1 * T:2 * T]
        WqT_s = wp[:, 2 * T:3 * T]
        WvaT_s = wp[:, 3 * T:4 * T]
        WvoT_s = wp[:, 4 * T:5 * T]
        ident_s = wp[:, 5 * T:6 * T]
        queryT = consts.tile([T, Q], BF16, tag="queryT")
        keyT = consts.tile([T, K], F32, tag="keyT")
        vT = consts.tile([T, KLOC], F32, tag="vT")
        simb = consts.tile([T, KLOC], F32, tag="simb")

        acc = accp.tile([T, Q], F32, tag="acc")

        # -------- early setup: projections (scoped psum pool) --------
        ss = ctx.enter_context(tc.tile_pool(name="sset", bufs=1))
        with tc.tile_pool(name="pearly", bufs=2, space="PSUM") as ps:
            kT_s = ss.tile([T, K], F32, tag="kT_s")
            nc.sync.dma_start(kT_s[:], kT)
            qT_s = ss.tile([T, Q], F32, tag="qT_s")
            nc.sync.dma_start(qT_s[:], qT)

            p2 = ps.tile([T, K], F32, tag="pbig")
            nc.tensor.matmul(p2[:, 0:T], WkT_s, kT_s[:, 0:T],
                             start=True, stop=True)
            nc.scalar.copy(keyT[:, 0:T], p2[:, 0:T])
            nc.tensor.matmul(p2[:, T:K], WkT_s, kT_s[:, T:K],
                             start=True, stop=True)
            nc.scalar.copy(keyT[:, T:K], p2[:, T:K])
            p1 = ps.tile([T, Q], F32, tag="pbig")
            nc.tensor.matmul(p1[:], WqT_s, qT_s[:], start=True, stop=True)
            nc.scalar.copy(queryT[:], p1[:])
            p3 = ps.tile([T, KLOC], F32, tag="pbig")
            nc.tensor.matmul(p3[:], WvaT_s, keyT[:, 0:KLOC], start=True, stop=True)
            nc.scalar.copy(vT[:], p3[:])

        def emit_sim(simp):
            # Gram + similarity chain; emitted under block 0 so its serial
            # latency hides behind the first block's streaming work.
            key_kt = ss.tile([T, 4, T], F32, tag="key_kt")
            kns = ss.tile([T, 4, T], F32, tag="kns")
            rn2 = ss.tile([T, 4], F32, tag="rn2")
            n2 = ss.tile([T, 4], F32, tag="n2")
            sqd = ss.tile([T, T], F32, tag="sqd")
            for c in range(4):
                pk = simp.tile([T, T], F32, tag="pbig")
                nc.tensor.matmul(pk[:], kT_s[:, c * T:(c + 1) * T], WkT_s,
                                 start=True, stop=True)
                nc.scalar.copy(key_kt[:, c, :], pk[:])
                nc.scalar.activation(sqd[:], key_kt[:, c, :], AF.Square,
                                     accum_out=n2[:, c:c + 1])
                nc.vector.reciprocal(rn2[:, c:c + 1], n2[:, c:c + 1])
                nc.vector.tensor_scalar(kns[:, c, :], key_kt[:, c, :],
                                        rn2[:, c:c + 1], None, ALU.mult)
            # Gram accumulates in the (still unused) acc bank; the first
            # mmacc has start=True which resets the bank afterwards
            for c in range(4):
                nc.tensor.matmul(acc[:, 0:T], kns[:, c, :], key_kt[:, c, :],
                                 start=(c == 0), stop=(c == 3))
            G_s = ss.tile([T, T], F32, tag="G_s")
            nc.scalar.copy(G_s[:], acc[:, 0:T])
            simc = ss.tile([T, 1], F32, tag="simc")
            sttd = ss.tile([T, T], F32, tag="sttd")
            ph = simp.tile([T, T], F32, tag="pbig")
            nc.tensor.matmul(ph[:], keyT[:, 0:T], G_s[:], start=True, stop=True)
            nc.vector.scalar_tensor_tensor(sttd[:], ph[:], rn2[:, 0:1],
                                           key_kt[:, 0, :], ALU.mult, ALU.mult,
                                           accum_out=simc[:])
            # transpose sim column -> row, then broadcast across partitions
            prow = simp.tile([1, KLOC], F32, tag="pbig")
            nc.tensor.matmul(prow[:], simc[:], ident_s, start=True, stop=True)
            simrow = ss.tile([1, KLOC], F32, tag="simrow")
            nc.scalar.copy(simrow[:], prow[:])
            onesr = ss.tile([1, T], F32, tag="onesr")
            nc.vector.memset(onesr[:], 1.0)
            pb = simp.tile([T, KLOC], F32, tag="pbig")
            nc.tensor.matmul(pb[:], onesr[:], simrow[:], start=True, stop=True)
            nc.scalar.copy(simb[:], pb[:])

        # ---------------- main loop over local keys ----------------
        # Variable-size blocks (groups of GROUP keys): large blocks amortize
        # fixed costs; small final blocks shallow out the pipeline tail.
        # Engine roles: PE mmL+mmacc, ACT exp+abs, DVE umul/max/denom,
        # GPSIMD lh/diag. Acc matmuls retire from a rolling queue one group
        # per front-group so they fill PE slack without delaying exps.
        NG = KLOC // GROUP
        SIZES = [8] * 7 + [4, 2, 2]
        assert sum(SIZES) == NG
        NB = len(SIZES)
        OFFS = [0]
        for s in SIZES:
            OFFS.append(OFFS[-1] + s)
        live = {}
        acc_queue = []

        lh_eng = nc.gpsimd
        dg_eng = nc.gpsimd

        def emit_lh(b):
            lhs = []
            for i in range(SIZES[b] * GROUP):
                j = OFFS[b] * GROUP + i
                lh = lhsp.tile([T, T], BF16, tag="lh")
                lw = T if st["lh"] else 8
                lh_eng.tensor_scalar(lh[:, 0:lw], WalT_s[:, 0:lw],
                                     keyT[:, j:j + 1], None, ALU.mult)
                lhs.append(lh)
            live[("lh", b)] = lhs

        def emit_acc_group(b, g):
            us, dgs = live[b]["us"], live[b]["dgs"]
            for i in range(GROUP):
                j = (OFFS[b] + g) * GROUP + i
                aw2 = Q if st["mmacc"] else 8
                nc.tensor.matmul(acc[:, 0:aw2], dgs[g * GROUP + i][:],
                                 us[g][:, i * Q:i * Q + aw2],
                                 start=(j == 0), stop=(j == KLOC - 1))
            if g == SIZES[b] - 1:
                del live[b]

        def emit_front(b):
            lhs = live.pop(("lh", b))
            nb = SIZES[b]
            Ls, es, us = [], [], []
            for g in range(nb):
                Lbig = Lp.tile([T, GROUP * Q], F32, tag="L")
                for i in range(GROUP):
                    mw = Q if st["mmL"] else 8
                    nc.tensor.matmul(Lbig[:, i * Q:i * Q + mw],
                                     lhs[g * GROUP + i][:],
                                     queryT[:, 0:mw], start=True, stop=True)
                Ls.append(Lbig)
                e = epool.tile([T, GROUP * Q], BF16, tag="e")
                ew = GROUP * Q if st["exp"] else 8
                nc.scalar.activation(e[:, 0:ew], Lbig[:, 0:ew], AF.Exp)
                es.append(e)
                if acc_queue:
                    emit_acc_group(*acc_queue.pop(0))
            bk = nb * GROUP
            for g in range(nb):
                u = upool.tile([T, GROUP * Q], BF16, tag="u")
                uw = GROUP * Q if st["umul"] else 8
                nc.vector.tensor_tensor(u[:, 0:uw], Ls[g][:, 0:uw],
                                        es[g][:, 0:uw], op=ALU.mult)
                us.append(u)
            sumabs = st_pool.tile([T, bk], F32, tag="sumabs")
            aw = Q if st["babs"] else 8
            ndve = max(0, round(0.19 * bk)) if b < NB - 1 else 0
            nk = 0
            for g in range(nb):
                for i in range(GROUP):
                    if nk < ndve:
                        nk += 1
                        # a slice of the |u| row-sums runs on DVE to balance
                        # the ACT/DVE load
                        nc.vector.tensor_reduce(
                            sumabs[:, g * GROUP + i:g * GROUP + i + 1],
                            us[g][:, i * Q:i * Q + aw].rearrange(
                                "p (g q) -> p g q", g=1),
                            axis=AX.X, op=ALU.add, apply_absolute_value=True)
                        continue
                    absd = apool.tile([T, Q], BF16, tag="absd")
                    nc.scalar.activation(absd[:, 0:aw],
                                         us[g][:, i * Q:i * Q + aw], AF.Abs,
                                         accum_out=sumabs[:, g * GROUP + i:
                                                          g * GROUP + i + 1])
            emax = st_pool.tile([T, bk], F32, tag="emax")
            xw = Q if st["bmax"] else 8
            for g in range(nb):
                # two-phase max: pairwise TT max (2x mode on bf16), then a
                # half-length reduce
                ev = es[g][:].rearrange("p (k h q) -> p k h q", k=GROUP, h=2)
                m1 = mpool.tile([T, GROUP * Q // 2], BF16, tag="m1")
                m1v = m1[:].rearrange("p (k q) -> p k q", k=GROUP)
                nc.vector.tensor_tensor(m1v[:, :, 0:xw // 2],
                                        ev[:, :, 0, 0:xw // 2],
                                        ev[:, :, 1, 0:xw // 2], op=ALU.max)
                nc.vector.tensor_reduce(
                    emax[:, g * GROUP:(g + 1) * GROUP],
                    m1v[:, :, 0:xw // 2], axis=AX.X, op=ALU.max)
            live[b] = dict(us=us, emax=emax, sumabs=sumabs)

        def emit_back(b, final=False):
            j0 = OFFS[b] * GROUP
            bk = SIZES[b] * GROUP
            emax, sumabs = live[b]["emax"], live[b]["sumabs"]
            fcol = st_pool.tile([T, bk], F32, tag="fcol")
            tt_eng = nc.vector if final else nc.gpsimd
            if st["tiny"]:
                d1 = st_pool.tile([T, bk], F32, tag="d1")
                tt_eng.tensor_tensor(d1[:], emax[:, 0:bk],
                                     simb[:, j0:j0 + bk], op=ALU.mult)
                d2 = st_pool.tile([T, bk], F32, tag="d2")
                tt_eng.tensor_tensor(d2[:], d1[:], sumabs[:, 0:bk], op=ALU.add)
                rd = st_pool.tile([T, bk], F32, tag="rd")
                nc.vector.reciprocal(rd[:], d2[:])
                tt_eng.tensor_tensor(fcol[:], rd[:],
                                     vT[:, j0:j0 + bk], op=ALU.mult)
            dgs = []
            us = live[b]["us"]
            for i in range(bk):
                dg = dgp.tile([T, T], BF16, tag="dg")
                dw = T if st["diag"] else 8
                eng = nc.vector if final else dg_eng
                eng.tensor_scalar(dg[:, 0:dw], WvoT_s[:, 0:dw],
                                  fcol[:, i:i + 1], None, ALU.mult)
                dgs.append(dg)
                if final:
                    j = j0 + i
                    aw2 = Q if st["mmacc"] else 8
                    nc.tensor.matmul(acc[:, 0:aw2], dg[:],
                                     us[i // GROUP][:, (i % GROUP) * Q:
                                                    (i % GROUP) * Q + aw2],
                                     start=(j == 0), stop=(j == KLOC - 1))
            live[b]["dgs"] = dgs

        with tc.tile_pool(name="lhs", bufs=2 * 16 + 2) as lhsp, \
             tc.tile_pool(name="ebuf", bufs=8 + 2) as epool, \
             tc.tile_pool(name="ubuf", bufs=3 * 8 + 2) as upool, \
             tc.tile_pool(name="absd", bufs=3) as apool, \
             tc.tile_pool(name="maxb", bufs=2) as mpool, \
             tc.tile_pool(name="stats", bufs=3) as st_pool, \
             tc.tile_pool(name="diag", bufs=2 * 16 + 2) as dgp, \
             tc.tile_pool(name="Lps", bufs=3, space="PSUM") as Lp, \
             tc.tile_pool(name="simp", bufs=1, space="PSUM") as simp:
            emit_lh(0)
            for b in range(NB):
                emit_front(b)
                if b == 1:
                    emit_sim(simp)
                if b + 1 < NB:
                    emit_lh(b + 1)
                if b >= 1:
                    emit_back(b - 1)
                    if b - 1 < NB - 1:
                        acc_queue.extend((b - 1, g) for g in range(SIZES[b - 1]))
            while acc_queue:
                emit_acc_group(*acc_queue.pop(0))
            emit_back(NB - 1, final=True)

        # ---------------- final: evacuate acc (already projected) ------
        with tc.tile_pool(name="fin", bufs=1) as fp:
            outS = fp.tile([T, Q], F32, tag="outS")
            nc.vector.tensor_copy(outS[:, 0:Q // 2], acc[:, 0:Q // 2])
            nc.sync.dma_start(outT[:, 0:Q // 2], outS[:, 0:Q // 2])
            nc.vector.tensor_copy(outS[:, Q // 2:Q], acc[:, Q // 2:Q])
            nc.sync.dma_start(outT[:, Q // 2:Q], outS[:, Q // 2:Q])

    nc.finalize()
    return nc


def _in_maps(query_tokens, key_tokens, Wk, Wq, Wva, Wal, Wvo):
    f = np.float32
    wpack = np.concatenate(
        [np.asarray(w).T.astype(f) for w in (Wk, Wal, Wq, Wva, Wvo)]
        + [np.eye(T, dtype=f)], axis=1)
    wts = {"wpack": np.ascontiguousarray(wpack)}
    maps = []
    for c in range(NCORES):
        b, r = c // 4, c % 4
        order = (np.arange(K) + r * KLOC) % K
        maps.append({
            "qT": np.ascontiguousarray(np.asarray(query_tokens)[b].T, dtype=f),
            "kT": np.ascontiguousarray(np.asarray(key_tokens)[b][order].T, dtype=f),
            **wts,
        })
    return maps


def kernel(query_tokens, key_tokens, Wk, Wq, Wva, Wal, Wvo):
    if "nc" not in _cache:
        _cache["nc"] = _build_program()
    nc = _cache["nc"]
    maps = _in_maps(query_tokens, key_tokens, Wk, Wq, Wva, Wal, Wvo)
    res = run_bass_kernel_spmd(nc, maps, core_ids=list(range(NCORES)))
    parts = [r["outT"] for r in res.results]
    out = np.stack(
        [(parts[4 * b] + parts[4 * b + 1] + parts[4 * b + 2] + parts[4 * b + 3]).T
         for b in range(B)]
    ).astype(np.float32)
    return out
